# revision 46
# baseline (speedup 1.0000x reference)
"""Trainium2 Bass kernel for ViTDet-style attention with decomposed
relative-position bias.

Problem shapes (hardcoded):
  x: (4, 32, 32, 768) f32, Wqkv: (768, 2304), Wproj: (768, 768),
  bproj: (768,), rel_pos_h/w: (63, 64).
  12 heads, head_dim 64, S = 32*32 = 1024.

Sharding: 48 (batch, head) pairs -> 6 heads per core, all of one batch per
core-pair. Each core computes its heads' attention and partial output
projections; the host sums the partials per batch and adds bproj.

Device algorithm per core (bf16 matmuls, fp32 PSUM accumulation):
  - coalesced input DMAs into per-tensor mega SBUF tiles (DMA issue on the
    sync queue costs ~650ns each, so few big DMAs beat many small ones)
  - v   = x @ Wv (natural layout, with an appended ones column per head)
  - qkT = Wqk^T @ x^T  (k pre-scaled on host), written into two mega-tiles
    kaug/qaug [128, 6*1024]
  - rel-pos bias rows DIRECTLY from qT: for shift s,
    BhT[r, q in h-block s] = sum_c rhT[c, s+r] * qT[c, q]  (all 6 heads in
    one N=192 matmul via a 3D moving AP; 4 shifts per PSUM tile, u-blocks
    padded to 256 f32 so no matmul output crosses a 2KB PSUM bank).
    These are interleaved between the k octiles so their PSUM->SBUF copies
    hide under qk matmuls.
  - scoresT (k x q) = kaug^T @ qaug in ONE K=128 matmul per tile:
    rows 0-63 = kT/qT, 64-95 = one-hot(h)/BhT, 96-127 = one-hot(w)/BwT
    => rel-pos bias folded into the QK matmul for free.
  - eT = exp(scoresT) on ScalarE from PSUM (attention is ScalarE-bound:
    48 exps x ~1.1us; two heads in flight keep ScalarE saturated).
  - avT (65 x q) accumulates v_aug^T over k blocks; row 64 = denominator.
  - normalize: av copied to SBUF (frees the PSUM ring early), then
    reciprocal_approx_fast + gpsimd partition-broadcast + gpsimd multiply.
  - proj in two passes: chunks 0,1 are projected DURING pair-2 attention
    (PE idles there) and written out as partial `out1`; chunk 2 after the
    last norm as `out2`. Host sums out1+out2 across the core pair + bproj.
"""

import numpy as np

import concourse.bass as bass
import concourse.bacc as bacc
import concourse.mybir as mybir
import concourse.tile as tile
from concourse.tile import add_dep_helper
from concourse.bass_utils import run_bass_kernel_spmd

F32 = mybir.dt.float32
BF16 = mybir.dt.bfloat16
EXP = mybir.ActivationFunctionType.Exp

NH = 12          # total heads
C = 768
HD = 64
H = W = 32
S = H * W        # 1024
B = 4
NCORES = 8
HPC = NH * B // NCORES   # heads per core = 6
NCH = 6                  # C // 128 input-channel chunks
NKB = S // 128           # 8 k blocks
NQB = S // 128           # 8 q blocks
NHALF = 512              # matmul moving-dim half
AW = HPC * S             # mega-tile width 6144


def build_program(dbg=False):
    nc = bacc.Bacc("TRN2", target_bir_lowering=False, debug=False)

    xT = nc.declare_dram_parameter("xT", [C, S], BF16, isOutput=False)
    wqk = nc.declare_dram_parameter("wqk", [C, 2 * HPC * HD], BF16, isOutput=False)
    wv = nc.declare_dram_parameter("wv", [C, HPC * HD], BF16, isOutput=False)
    wproj = nc.declare_dram_parameter("wproj", [HPC * HD, C], BF16, isOutput=False)
    rhT = nc.declare_dram_parameter("rhT", [HD, 2 * H - 1], BF16, isOutput=False)
    rwT = nc.declare_dram_parameter("rwT", [HD, 2 * W - 1], BF16, isOutput=False)
    oh64 = nc.declare_dram_parameter("oh64", [64, AW], BF16, isOutput=False)
    ones = nc.declare_dram_parameter("ones", [1, S], BF16, isOutput=False)
    out1 = nc.declare_dram_parameter("out1", [S, C], F32, isOutput=True)
    out2 = nc.declare_dram_parameter("out2", [S, C], F32, isOutput=True)

    with tile.TileContext(nc) as tc:
        with (
            tc.tile_pool(name="persist", bufs=1) as persist,
            tc.tile_pool(name="psc", bufs=2, space="PSUM") as psc,
            tc.tile_pool(name="pav", bufs=2, space="PSUM") as pav,
            tc.tile_pool(name="et", bufs=4) as et_pool,
            tc.tile_pool(name="small", bufs=2) as small,
        ):
            # ---- coalesced persistent loads, in consumption order ----
            xT_sb = persist.tile([128, NCH * S], BF16, tag="xT", name="xT_sb")
            wv_sb = persist.tile([128, NCH * HPC * HD], BF16, tag="wv", name="wv_sb")
            wqk_sb = persist.tile([128, NCH * 2 * HPC * HD], BF16, tag="wqk",
                                  name="wqk_sb")
            wproj_sb = persist.tile([128, 3 * C], BF16, tag="wproj",
                                    name="wproj_sb")
            ones_sb = persist.tile([1, S], BF16, tag="ones", name="ones_sb")
            rhT_sb = persist.tile([HD, 2 * H - 1], BF16, tag="rhT", name="rhT_sb")
            rwT_sb = persist.tile([HD, 2 * W - 1], BF16, tag="rwT", name="rwT_sb")
            kaug = persist.tile([128, AW], BF16, tag="kaug", name="kaug")
            qaug = persist.tile([128, AW], BF16, tag="qaug", name="qaug")

            # wv first and xT in thirds so the v matmuls start early
            nc.sync.dma_start(
                wv_sb[:],
                bass.AP(wv, 0, [[HPC * HD, 128], [128 * HPC * HD, NCH],
                                [1, HPC * HD]]))
            for third in range(3):
                nc.sync.dma_start(
                    xT_sb[:, 2 * third * S:2 * (third + 1) * S],
                    bass.AP(xT, 2 * third * 128 * S,
                            [[S, 128], [128 * S, 2], [1, S]]))
            nc.sync.dma_start(ones_sb[:], ones[:, :])
            nc.sync.dma_start(
                wqk_sb[:],
                bass.AP(wqk, 0, [[2 * HPC * HD, 128],
                                 [128 * 2 * HPC * HD, NCH], [1, 2 * HPC * HD]]))
            nc.sync.dma_start(kaug[64:128, :], oh64[:, :])
            nc.sync.dma_start(rhT_sb[:], rhT[:, :])
            nc.sync.dma_start(rwT_sb[:], rwT[:, :])
            nc.sync.dma_start(
                wproj_sb[:],
                bass.AP(wproj, 0, [[C, 128], [128 * C, 3], [1, C]]))

            def xs(ci, lo, hi):
                return xT_sb[:, S * ci + lo:S * ci + hi]

            # ---- v projection (natural) + ones column ----
            # v_sb[sb]: (128, 6*65) cols [65i..65i+64) = head i v, col 65i+64 = 1
            # Manual (multi-dim strided) APs get imprecise subtile dep ranges,
            # so ordering edges for their readers are added via add_dep_helper
            # (engine program order covers the rest).
            vcopy_insts = []
            v_sb = [persist.tile([128, HPC * (HD + 1)], BF16, tag=f"v{sb}",
                                 name=f"v{sb}") for sb in range(NKB)]
            for sb in range(NKB):
                vp = psc.tile([128, HPC * HD + HPC], F32, tag="big", name="vp")
                for ci in range(NCH):
                    nc.tensor.matmul(
                        vp[:, 0:HPC * HD],
                        xs(ci, 128 * sb, 128 * (sb + 1)),
                        wv_sb[:, HPC * HD * ci:HPC * HD * (ci + 1)],
                        start=(ci == 0), stop=(ci == NCH - 1))
                nc.tensor.matmul(vp[:, HPC * HD:HPC * HD + HPC],
                                 ones_sb[0:1, 128 * sb:128 * (sb + 1)],
                                 ones_sb[0:1, 0:HPC], start=True, stop=True)
                src = bass.AP(vp.tensor, vp[:].offset,
                              [vp[:].ap[0], [HD, HPC], [1, HD]])
                dst = bass.AP(v_sb[sb].tensor, v_sb[sb][:].offset,
                              [v_sb[sb][:].ap[0], [HD + 1, HPC], [1, HD]])
                ones_src = bass.AP(vp.tensor, vp[:].offset + HPC * HD,
                                   [vp[:].ap[0], [1, HPC]])
                ones_dst = bass.AP(v_sb[sb].tensor, v_sb[sb][:].offset + HD,
                                   [v_sb[sb][:].ap[0], [HD + 1, HPC]])
                if sb % 2:
                    vcopy_insts.append(nc.scalar.copy(dst, src))
                    vcopy_insts.append(nc.scalar.copy(ones_dst, ones_src))
                else:
                    vcopy_insts.append(nc.vector.tensor_copy(dst, src))
                    vcopy_insts.append(nc.vector.tensor_copy(ones_dst, ones_src))

            # ---- qk projection into the mega-tiles ----
            # octile t covers oc rows [128t, 128t+128): t<3 -> q, t>=3 -> k
            qcopy_insts = []

            def qk_octile(t):
                qp = psc.tile([128, S], F32, tag="big", name="qp")
                for ci in range(NCH):
                    for nh in range(S // NHALF):
                        nc.tensor.matmul(
                            qp[:, NHALF * nh:NHALF * (nh + 1)],
                            wqk_sb[:, 2 * HPC * HD * ci + 128 * t:
                                   2 * HPC * HD * ci + 128 * (t + 1)],
                            xs(ci, NHALF * nh, NHALF * (nh + 1)),
                            start=(ci == 0), stop=(ci == NCH - 1))
                for sub in range(2):
                    head = (t % 3) * 2 + sub
                    dstt = qaug if t < 3 else kaug
                    dst_ap = dstt[0:64, S * head:S * (head + 1)]
                    src_ap = qp[64 * sub:64 * sub + 64, :]
                    if sub == 0:
                        cp = nc.scalar.copy(dst_ap, src_ap)
                    else:
                        cp = nc.vector.tensor_copy(dst_ap, src_ap)
                    if t < 3:
                        qcopy_insts.append(cp)

            # ---- rel-pos bias rows, direct from qT ----
            # For shift s, rows r in [0,32):
            #   qaug[64+r, q in block s of head i] = sum_c tbl[c, s+r]*qT[c, q]
            qa64 = qaug[0:64, 0:1]
            band_copy_insts = []
            band_state = {"first": None, "eng": 0}

            def band_axis(axis):
                # 4 shift-quad groups packed vertically (partition offsets
                # 0/32/64/96) into each [128, 1024] PSUM tile: the copy of
                # group g only gates the matmuls 8 groups later, so the
                # PSUM->SBUF copies hide under the k octile matmuls.
                tbl = rhT_sb if axis == 0 else rwT_sb
                for grp in range(3):   # groups of 3 shift-quads (last has 2)
                    bp = psc.tile([128, S], F32, tag="big", name="bp")
                    for j in range(3 if grp < 2 else 2):
                        sq = 3 * grp + j
                        prow = bp[32 * j:32 * j + 32, 0:1]
                        for u in range(4):
                            s = 4 * sq + u
                            if axis == 0:
                                # h-block of head i: cols i*S + 32*s + jj
                                rhs = bass.AP(qaug.tensor, qa64.offset + 32 * s,
                                              [qa64.ap[0], [S, HPC], [1, 32]])
                            else:
                                # w-block: cols i*S + s + 32*jh
                                rhs = bass.AP(qaug.tensor, qa64.offset + s,
                                              [qa64.ap[0], [S, HPC], [32, 32]])
                            mm = nc.tensor.matmul(
                                bass.AP(bp.tensor, prow.offset + 256 * u,
                                        [prow.ap[0], [1, 192]]),
                                tbl[:, s:s + 32], rhs,
                                start=True, stop=True)
                            if band_state["first"] is None:
                                band_state["first"] = mm
                                for cp in qcopy_insts:
                                    add_dep_helper(mm.ins, cp.ins, sync=True,
                                                   reason="band mm reads qT")
                        if axis == 0:
                            dst = bass.AP(qaug.tensor,
                                          qaug[64:96, 0:1].offset + 128 * sq,
                                          [qaug[64:96, 0:1].ap[0],
                                           [S, HPC], [32, 4], [1, 32]])
                            src = bass.AP(bp.tensor, prow.offset,
                                          [prow.ap[0], [32, HPC], [256, 4],
                                           [1, 32]])
                        else:
                            dst = bass.AP(qaug.tensor,
                                          qaug[96:128, 0:1].offset + 4 * sq,
                                          [qaug[96:128, 0:1].ap[0],
                                           [S, HPC], [32, 32], [1, 4]])
                            src = bass.AP(bp.tensor, prow.offset,
                                          [prow.ap[0], [32, HPC], [1, 32],
                                           [256, 4]])
                        if band_state["eng"] % 2:
                            band_copy_insts.append(nc.scalar.copy(dst, src))
                        else:
                            band_copy_insts.append(
                                nc.vector.tensor_copy(dst, src))
                        band_state["eng"] += 1

            # band matmuls slotted between k octiles; copies drain alongside
            for t in range(5):
                qk_octile(t)
            band_axis(0)
            qk_octile(5)
            band_axis(1)

            if dbg:
                qaug_dbg = nc.declare_dram_parameter("qaug_dbg", [128, AW], BF16,
                                                     isOutput=True)
                kaug_dbg = nc.declare_dram_parameter("kaug_dbg", [128, AW], BF16,
                                                     isOutput=True)
                vsb_dbg = nc.declare_dram_parameter("vsb_dbg",
                                                    [128, NKB * HPC * (HD + 1)],
                                                    BF16, isOutput=True)
                d1 = nc.sync.dma_start(qaug_dbg[:, :], qaug[:, :])
                for cp in band_copy_insts:
                    add_dep_helper(d1.ins, cp.ins, sync=True, reason="dbg")
                nc.sync.dma_start(kaug_dbg[:, :], kaug[:, :])
                for sb in range(NKB):
                    d2 = nc.sync.dma_start(
                        vsb_dbg[:, sb * 390:(sb + 1) * 390], v_sb[sb][:])
                    for cp in vcopy_insts:
                        add_dep_helper(d2.ins, cp.ins, sync=True, reason="dbg")

            # ---- attention, two heads in flight per pair ----
            out_hT = [persist.tile([128, S], BF16, tag=f"ohT{c}", name=f"ohT{c}")
                      for c in range(HPC * HD // 128)]
            state = {"first_sc": True, "first_av": True}

            def sc_mms(head, kb):
                scp = psc.tile([128, S], F32, tag="big", name="scp")
                for nh in range(S // NHALF):
                    mm = nc.tensor.matmul(
                        scp[:, NHALF * nh:NHALF * (nh + 1)],
                        kaug[:, S * head + 128 * kb:S * head + 128 * (kb + 1)],
                        qaug[:, S * head + NHALF * nh:S * head + NHALF * (nh + 1)],
                        start=True, stop=True)
                    if state["first_sc"]:
                        state["first_sc"] = False
                        for cp in band_copy_insts:
                            add_dep_helper(mm.ins, cp.ins, sync=True,
                                           reason="scores read band rows")
                return scp

            def av_mms(av, head, kb, e):
                for nh in range(S // NHALF):
                    mm = nc.tensor.matmul(
                        av[:, NHALF * nh:NHALF * (nh + 1)],
                        v_sb[kb][:, (HD + 1) * head:(HD + 1) * (head + 1)],
                        e[:, NHALF * nh:NHALF * (nh + 1)],
                        start=(kb == 0), stop=(kb == NKB - 1))
                    if state["first_av"]:
                        state["first_av"] = False
                        for cp in vcopy_insts:
                            add_dep_helper(mm.ins, cp.ins, sync=True,
                                           reason="av reads v_sb")

            def norm(av, head, last=False):
                # the reciprocal input must sit at partition 0 (the custom
                # DVE op mishandles partition-offset inputs), hence drow
                drow = small.tile([1, S], F32, tag="drow", name="drow", bufs=2)
                if last:
                    # ScalarE is free at the tail; keep DVE for recip+mult
                    nc.scalar.copy(drow[:], av[HD:HD + 1, :])
                else:
                    nc.vector.tensor_copy(drow[:], av[HD:HD + 1, :])
                # copy av to SBUF: frees the PSUM ring slot early (for the
                # next pair's av matmuls, or for pass-2 psum reuse at the tail)
                avsb = small.tile([HD + 1, S], F32, tag="avsb",
                                  name="avsb", bufs=2)
                nc.vector.tensor_copy(avsb[:], av[:])
                avsrc = avsb[0:HD, :]
                rrow = small.tile([1, S], F32, tag="recip", name="rrow", bufs=2)
                nc.vector.reciprocal_approx_fast(rrow[:], drow[:])
                rb = small.tile([64, S], F32, tag="rbcast", name="rb", bufs=2)
                nc.gpsimd.partition_broadcast(rb[:], rrow[:])
                chunk, row = head // 2, (head % 2) * 64
                nc.vector.tensor_tensor(
                    out_hT[chunk][row:row + 64, :], avsrc, rb[:],
                    op=mybir.AluOpType.mult)

            def proj_pass1(qb, scalar_copy=False):
                # chunks 0,1 of the projection, run during pair-2 attention
                pp1 = psc.tile([128, C], F32, tag="big", name="pp1")
                for ci in (0, 1):
                    nc.tensor.matmul(
                        pp1[:, 0:NHALF],
                        out_hT[ci][:, 128 * qb:128 * (qb + 1)],
                        wproj_sb[:, C * ci:C * ci + NHALF],
                        start=(ci == 0), stop=(ci == 1))
                    nc.tensor.matmul(
                        pp1[:, NHALF:C],
                        out_hT[ci][:, 128 * qb:128 * (qb + 1)],
                        wproj_sb[:, C * ci + NHALF:C * (ci + 1)],
                        start=(ci == 0), stop=(ci == 1))
                pp_sb = small.tile([128, C], F32, tag="pp_sb", name="pp_sb",
                                   bufs=4)
                if scalar_copy:
                    nc.scalar.copy(pp_sb[:], pp1[:])
                else:
                    nc.vector.tensor_copy(pp_sb[:], pp1[:])
                nc.sync.dma_start(out1[128 * qb:128 * (qb + 1), :], pp_sb[:])

            for p in range(HPC // 2):
                ha, hb = 2 * p, 2 * p + 1
                av_a = pav.tile([HD + 1, S], F32, tag="av", name="av_a")
                av_b = pav.tile([HD + 1, S], F32, tag="av", name="av_b")
                ea = eb = ebprev = None
                for kb in range(NKB + 1):
                    if kb < NKB:
                        sca = sc_mms(ha, kb)
                        scb = sc_mms(hb, kb)
                    if kb > 0:
                        av_mms(av_a, ha, kb - 1, ea)
                    if kb > 1:
                        av_mms(av_b, hb, kb - 2, ebprev)
                    if kb < NKB:
                        ea = et_pool.tile([128, S], BF16, tag="et", name="ea")
                        nc.scalar.activation(ea[:], sca[:], EXP)
                        ebprev = eb
                        eb = et_pool.tile([128, S], BF16, tag="et", name="eb")
                        nc.scalar.activation(eb[:], scb[:], EXP)
                    if p == 2 and 2 <= kb:
                        proj_pass1(kb - 2)
                norm(av_a, ha, last=(p == 2))
                av_mms(av_b, hb, NKB - 1, eb)
                if p == 2:
                    proj_pass1(6, scalar_copy=True)
                    proj_pass1(7, scalar_copy=True)
                norm(av_b, hb, last=(p == 2))

            # ---- chunk-2 projection after the last norm ----
            # alternate between the now-free "av" and "big" PSUM rings for an
            # effective ring depth of 4 in this copy-gated pipeline
            for qb in range(NQB):
                pool, tg = (pav, "av") if qb % 2 else (psc, "big")
                pp2 = pool.tile([128, C], F32, tag=tg, name="pp2")
                nc.tensor.matmul(
                    pp2[:, 0:NHALF],
                    out_hT[2][:, 128 * qb:128 * (qb + 1)],
                    wproj_sb[:, 2 * C:2 * C + NHALF],
                    start=True, stop=True)
                nc.tensor.matmul(
                    pp2[:, NHALF:C],
                    out_hT[2][:, 128 * qb:128 * (qb + 1)],
                    wproj_sb[:, 2 * C + NHALF:3 * C],
                    start=True, stop=True)
                pp_sb2 = small.tile([128, C], F32, tag="pp_sb2", name="pp_sb2",
                                    bufs=4)
                (nc.scalar.copy if qb % 2 else nc.vector.tensor_copy)(
                    pp_sb2[:], pp2[:])
                nc.sync.dma_start(out2[128 * qb:128 * (qb + 1), :], pp_sb2[:])

    nc.compile()
    return nc


def shard_inputs(x, Wqkv, Wproj, rel_pos_h, rel_pos_w):
    """Build the 8 per-core input maps."""
    import ml_dtypes
    bf16 = ml_dtypes.bfloat16
    scale = HD ** (-0.5)
    x = np.asarray(x, dtype=np.float32)
    Wqkv = np.asarray(Wqkv, dtype=np.float32)
    Wproj = np.asarray(Wproj, dtype=np.float32)
    rhT = np.ascontiguousarray(np.asarray(rel_pos_h, np.float32).T).astype(bf16)
    rwT = np.ascontiguousarray(np.asarray(rel_pos_w, np.float32).T).astype(bf16)
    oh = np.zeros((64, S), np.float32)
    for khp in range(H):
        oh[khp, (31 - khp) * W:(31 - khp) * W + W] = 1.0
    for kwp in range(W):
        oh[32 + kwp, 31 - kwp::W] = 1.0
    oh64 = np.ascontiguousarray(np.tile(oh, (1, HPC))).astype(bf16)
    ones = np.ones((1, S), np.float32).astype(bf16)
    in_maps = []
    for core in range(NCORES):
        b = core // 2
        h0 = (core % 2) * HPC
        xb = x[b].reshape(S, C)
        xT = np.ascontiguousarray(xb.T).astype(bf16)
        wq = Wqkv[:, h0 * HD:(h0 + HPC) * HD]
        wk = Wqkv[:, C + h0 * HD:C + (h0 + HPC) * HD] * scale
        wqk = np.ascontiguousarray(np.concatenate([wq, wk], axis=1)).astype(bf16)
        wv = np.ascontiguousarray(
            Wqkv[:, 2 * C + h0 * HD:2 * C + (h0 + HPC) * HD]).astype(bf16)
        wp = np.ascontiguousarray(Wproj[h0 * HD:(h0 + HPC) * HD, :]).astype(bf16)
        in_maps.append({"xT": xT, "wqk": wqk, "wv": wv, "wproj": wp,
                        "rhT": rhT, "rwT": rwT, "oh64": oh64, "ones": ones})
    return in_maps


def combine(results, bproj):
    """Sum the per-core partials into the full (B, H, W, C) output."""
    bproj = np.asarray(bproj, dtype=np.float32)
    out = np.empty((B, H, W, C), dtype=np.float32)
    for b in range(B):
        acc = bproj.copy()
        for core in (2 * b, 2 * b + 1):
            acc = acc + results[core]["out1"] + results[core]["out2"]
        out[b] = acc.reshape(H, W, C)
    return out


_NC_CACHE = {}


def kernel(x, Wqkv, Wproj, bproj, rel_pos_h, rel_pos_w):
    if "nc" not in _NC_CACHE:
        _NC_CACHE["nc"] = build_program()
    nc = _NC_CACHE["nc"]
    in_maps = shard_inputs(x, Wqkv, Wproj, rel_pos_h, rel_pos_w)
    res = run_bass_kernel_spmd(nc, in_maps, list(range(NCORES)))
    return combine(res.results, bproj)


# revision 47
# speedup vs baseline: 1.0261x; 1.0261x over previous
"""Trainium2 Bass kernel for ViTDet-style attention with decomposed
relative-position bias.

Problem shapes (hardcoded):
  x: (4, 32, 32, 768) f32, Wqkv: (768, 2304), Wproj: (768, 768),
  bproj: (768,), rel_pos_h/w: (63, 64).
  12 heads, head_dim 64, S = 32*32 = 1024.

Sharding: 48 (batch, head) pairs -> 6 heads per core, all of one batch per
core-pair. Each core computes its heads' attention and partial output
projections; the host sums the partials per batch and adds bproj.

Device algorithm per core (bf16 matmuls, fp32 PSUM accumulation):
  - coalesced input DMAs into per-tensor mega SBUF tiles (DMA issue on the
    sync queue costs ~650ns each, so few big DMAs beat many small ones)
  - v   = x @ Wv (natural layout, with an appended ones column per head)
  - qkT = Wqk^T @ x^T  (k pre-scaled on host), written into two mega-tiles
    kaug/qaug [128, 6*1024]
  - rel-pos bias rows DIRECTLY from qT: for shift s,
    BhT[r, q in h-block s] = sum_c rhT[c, s+r] * qT[c, q]  (all 6 heads in
    one N=192 matmul via a 3D moving AP; 4 shifts per PSUM tile, u-blocks
    padded to 256 f32 so no matmul output crosses a 2KB PSUM bank).
    These are interleaved between the k octiles so their PSUM->SBUF copies
    hide under qk matmuls.
  - scoresT (k x q) = kaug^T @ qaug in ONE K=128 matmul per tile:
    rows 0-63 = kT/qT, 64-95 = one-hot(h)/BhT, 96-127 = one-hot(w)/BwT
    => rel-pos bias folded into the QK matmul for free.
  - eT = exp(scoresT) on ScalarE from PSUM (attention is ScalarE-bound:
    48 exps x ~1.1us; two heads in flight keep ScalarE saturated).
  - avT (65 x q) accumulates v_aug^T over k blocks; row 64 = denominator.
  - normalize: av copied to SBUF (frees the PSUM ring early), then
    reciprocal_approx_fast + gpsimd partition-broadcast + gpsimd multiply.
  - proj in two passes: chunks 0,1 are projected DURING pair-2 attention
    (PE idles there) and written out as partial `out1`; chunk 2 after the
    last norm as `out2`. Host sums out1+out2 across the core pair + bproj.
"""

import numpy as np

import concourse.bass as bass
import concourse.bacc as bacc
import concourse.mybir as mybir
import concourse.tile as tile
from concourse.tile import add_dep_helper
from concourse.bass_utils import run_bass_kernel_spmd

F32 = mybir.dt.float32
BF16 = mybir.dt.bfloat16
EXP = mybir.ActivationFunctionType.Exp

NH = 12          # total heads
C = 768
HD = 64
H = W = 32
S = H * W        # 1024
B = 4
NCORES = 8
HPC = NH * B // NCORES   # heads per core = 6
NCH = 6                  # C // 128 input-channel chunks
NKB = S // 128           # 8 k blocks
NQB = S // 128           # 8 q blocks
NHALF = 512              # matmul moving-dim half
AW = HPC * S             # mega-tile width 6144


def build_program(dbg=False):
    nc = bacc.Bacc("TRN2", target_bir_lowering=False, debug=False)

    xT = nc.declare_dram_parameter("xT", [C, S], BF16, isOutput=False)
    wqk = nc.declare_dram_parameter("wqk", [C, 2 * HPC * HD], BF16, isOutput=False)
    wv = nc.declare_dram_parameter("wv", [C, HPC * HD], BF16, isOutput=False)
    wproj = nc.declare_dram_parameter("wproj", [HPC * HD, C], BF16, isOutput=False)
    rhT = nc.declare_dram_parameter("rhT", [HD, 2 * H - 1], BF16, isOutput=False)
    rwT = nc.declare_dram_parameter("rwT", [HD, 2 * W - 1], BF16, isOutput=False)
    oh64 = nc.declare_dram_parameter("oh64", [64, AW], BF16, isOutput=False)
    ones = nc.declare_dram_parameter("ones", [1, S], BF16, isOutput=False)
    out1 = nc.declare_dram_parameter("out1", [S, C], F32, isOutput=True)
    out2 = nc.declare_dram_parameter("out2", [S, C], F32, isOutput=True)

    with tile.TileContext(nc) as tc:
        with (
            tc.tile_pool(name="persist", bufs=1) as persist,
            tc.tile_pool(name="psc", bufs=2, space="PSUM") as psc,
            tc.tile_pool(name="pav", bufs=2, space="PSUM") as pav,
            tc.tile_pool(name="et", bufs=4) as et_pool,
            tc.tile_pool(name="small", bufs=2) as small,
        ):
            # ---- coalesced persistent loads, in consumption order ----
            xT_sb = persist.tile([128, NCH * S], BF16, tag="xT", name="xT_sb")
            wv_sb = persist.tile([128, NCH * HPC * HD], BF16, tag="wv", name="wv_sb")
            wqk_sb = persist.tile([128, NCH * 2 * HPC * HD], BF16, tag="wqk",
                                  name="wqk_sb")
            wproj_sb = persist.tile([128, 3 * C], BF16, tag="wproj",
                                    name="wproj_sb")
            ones_sb = persist.tile([1, S], BF16, tag="ones", name="ones_sb")
            rhT_sb = persist.tile([HD, 2 * H - 1], BF16, tag="rhT", name="rhT_sb")
            rwT_sb = persist.tile([HD, 2 * W - 1], BF16, tag="rwT", name="rwT_sb")
            kaug = persist.tile([128, AW], BF16, tag="kaug", name="kaug")
            qaug = persist.tile([128, AW], BF16, tag="qaug", name="qaug")

            # wv first and xT in thirds so the v matmuls start early
            nc.sync.dma_start(
                wv_sb[:],
                bass.AP(wv, 0, [[HPC * HD, 128], [128 * HPC * HD, NCH],
                                [1, HPC * HD]]))
            for third in range(3):
                nc.sync.dma_start(
                    xT_sb[:, 2 * third * S:2 * (third + 1) * S],
                    bass.AP(xT, 2 * third * 128 * S,
                            [[S, 128], [128 * S, 2], [1, S]]))
            nc.sync.dma_start(ones_sb[:], ones[:, :])
            nc.sync.dma_start(
                wqk_sb[:],
                bass.AP(wqk, 0, [[2 * HPC * HD, 128],
                                 [128 * 2 * HPC * HD, NCH], [1, 2 * HPC * HD]]))
            nc.sync.dma_start(kaug[64:128, :], oh64[:, :])
            nc.sync.dma_start(rhT_sb[:], rhT[:, :])
            nc.sync.dma_start(rwT_sb[:], rwT[:, :])
            nc.sync.dma_start(
                wproj_sb[:],
                bass.AP(wproj, 0, [[C, 128], [128 * C, 3], [1, C]]))

            def xs(ci, lo, hi):
                return xT_sb[:, S * ci + lo:S * ci + hi]

            # ---- v projection (natural) + ones column ----
            # v_sb[sb]: (128, 6*65) cols [65i..65i+64) = head i v, col 65i+64 = 1
            # Manual (multi-dim strided) APs get imprecise subtile dep ranges,
            # so ordering edges for their readers are added via add_dep_helper
            # (engine program order covers the rest).
            vcopy_insts = []
            v_sb = [persist.tile([128, HPC * (HD + 1)], BF16, tag=f"v{sb}",
                                 name=f"v{sb}") for sb in range(NKB)]
            for sb in range(NKB):
                vp = psc.tile([128, HPC * HD + HPC], F32, tag="big", name="vp")
                for ci in range(NCH):
                    nc.tensor.matmul(
                        vp[:, 0:HPC * HD],
                        xs(ci, 128 * sb, 128 * (sb + 1)),
                        wv_sb[:, HPC * HD * ci:HPC * HD * (ci + 1)],
                        start=(ci == 0), stop=(ci == NCH - 1))
                nc.tensor.matmul(vp[:, HPC * HD:HPC * HD + HPC],
                                 ones_sb[0:1, 128 * sb:128 * (sb + 1)],
                                 ones_sb[0:1, 0:HPC], start=True, stop=True)
                src = bass.AP(vp.tensor, vp[:].offset,
                              [vp[:].ap[0], [HD, HPC], [1, HD]])
                dst = bass.AP(v_sb[sb].tensor, v_sb[sb][:].offset,
                              [v_sb[sb][:].ap[0], [HD + 1, HPC], [1, HD]])
                ones_src = bass.AP(vp.tensor, vp[:].offset + HPC * HD,
                                   [vp[:].ap[0], [1, HPC]])
                ones_dst = bass.AP(v_sb[sb].tensor, v_sb[sb][:].offset + HD,
                                   [v_sb[sb][:].ap[0], [HD + 1, HPC]])
                if sb % 2:
                    vcopy_insts.append(nc.scalar.copy(dst, src))
                    vcopy_insts.append(nc.scalar.copy(ones_dst, ones_src))
                else:
                    vcopy_insts.append(nc.vector.tensor_copy(dst, src))
                    vcopy_insts.append(nc.vector.tensor_copy(ones_dst, ones_src))

            # ---- qk projection into the mega-tiles ----
            # octile t covers oc rows [128t, 128t+128): t<3 -> q, t>=3 -> k
            qcopy_insts = []

            def qk_octile(t):
                qp = psc.tile([128, S], F32, tag="big", name="qp")
                for ci in range(NCH):
                    for nh in range(S // NHALF):
                        nc.tensor.matmul(
                            qp[:, NHALF * nh:NHALF * (nh + 1)],
                            wqk_sb[:, 2 * HPC * HD * ci + 128 * t:
                                   2 * HPC * HD * ci + 128 * (t + 1)],
                            xs(ci, NHALF * nh, NHALF * (nh + 1)),
                            start=(ci == 0), stop=(ci == NCH - 1))
                for sub in range(2):
                    head = (t % 3) * 2 + sub
                    dstt = qaug if t < 3 else kaug
                    dst_ap = dstt[0:64, S * head:S * (head + 1)]
                    src_ap = qp[64 * sub:64 * sub + 64, :]
                    if sub == 0:
                        cp = nc.scalar.copy(dst_ap, src_ap)
                    else:
                        cp = nc.vector.tensor_copy(dst_ap, src_ap)
                    if t < 3:
                        qcopy_insts.append(cp)

            # ---- rel-pos bias rows, direct from qT ----
            # For shift s, rows r in [0,32):
            #   qaug[64+r, q in block s of head i] = sum_c tbl[c, s+r]*qT[c, q]
            qa64 = qaug[0:64, 0:1]
            band_copy_insts = []
            band_state = {"first": None, "eng": 0}

            def band_axis(axis):
                # 4 shift-quad groups packed vertically (partition offsets
                # 0/32/64/96) into each [128, 1024] PSUM tile: the copy of
                # group g only gates the matmuls 8 groups later, so the
                # PSUM->SBUF copies hide under the k octile matmuls.
                tbl = rhT_sb if axis == 0 else rwT_sb
                for grp in range(3):   # groups of 3 shift-quads (last has 2)
                    bp = psc.tile([128, S], F32, tag="big", name="bp")
                    for j in range(3 if grp < 2 else 2):
                        sq = 3 * grp + j
                        prow = bp[32 * j:32 * j + 32, 0:1]
                        for u in range(4):
                            s = 4 * sq + u
                            if axis == 0:
                                # h-block of head i: cols i*S + 32*s + jj
                                rhs = bass.AP(qaug.tensor, qa64.offset + 32 * s,
                                              [qa64.ap[0], [S, HPC], [1, 32]])
                            else:
                                # w-block: cols i*S + s + 32*jh
                                rhs = bass.AP(qaug.tensor, qa64.offset + s,
                                              [qa64.ap[0], [S, HPC], [32, 32]])
                            mm = nc.tensor.matmul(
                                bass.AP(bp.tensor, prow.offset + 256 * u,
                                        [prow.ap[0], [1, 192]]),
                                tbl[:, s:s + 32], rhs,
                                start=True, stop=True)
                            if band_state["first"] is None:
                                band_state["first"] = mm
                                for cp in qcopy_insts:
                                    add_dep_helper(mm.ins, cp.ins, sync=True,
                                                   reason="band mm reads qT")
                        if axis == 0:
                            dst = bass.AP(qaug.tensor,
                                          qaug[64:96, 0:1].offset + 128 * sq,
                                          [qaug[64:96, 0:1].ap[0],
                                           [S, HPC], [32, 4], [1, 32]])
                            src = bass.AP(bp.tensor, prow.offset,
                                          [prow.ap[0], [32, HPC], [256, 4],
                                           [1, 32]])
                        else:
                            dst = bass.AP(qaug.tensor,
                                          qaug[96:128, 0:1].offset + 4 * sq,
                                          [qaug[96:128, 0:1].ap[0],
                                           [S, HPC], [32, 32], [1, 4]])
                            src = bass.AP(bp.tensor, prow.offset,
                                          [prow.ap[0], [32, HPC], [1, 32],
                                           [256, 4]])
                        if band_state["eng"] % 2:
                            band_copy_insts.append(nc.scalar.copy(dst, src))
                        else:
                            band_copy_insts.append(
                                nc.vector.tensor_copy(dst, src))
                        band_state["eng"] += 1

            # band matmuls slotted between k octiles; copies drain alongside
            for t in range(5):
                qk_octile(t)
            band_axis(0)
            qk_octile(5)
            band_axis(1)

            if dbg:
                qaug_dbg = nc.declare_dram_parameter("qaug_dbg", [128, AW], BF16,
                                                     isOutput=True)
                kaug_dbg = nc.declare_dram_parameter("kaug_dbg", [128, AW], BF16,
                                                     isOutput=True)
                vsb_dbg = nc.declare_dram_parameter("vsb_dbg",
                                                    [128, NKB * HPC * (HD + 1)],
                                                    BF16, isOutput=True)
                d1 = nc.sync.dma_start(qaug_dbg[:, :], qaug[:, :])
                for cp in band_copy_insts:
                    add_dep_helper(d1.ins, cp.ins, sync=True, reason="dbg")
                nc.sync.dma_start(kaug_dbg[:, :], kaug[:, :])
                for sb in range(NKB):
                    d2 = nc.sync.dma_start(
                        vsb_dbg[:, sb * 390:(sb + 1) * 390], v_sb[sb][:])
                    for cp in vcopy_insts:
                        add_dep_helper(d2.ins, cp.ins, sync=True, reason="dbg")

            # ---- attention, two heads in flight per pair ----
            out_hT = [persist.tile([128, S], BF16, tag=f"ohT{c}", name=f"ohT{c}")
                      for c in range(HPC * HD // 128)]
            state = {"first_sc": True, "first_av": True}

            def sc_mms(head, kb):
                scp = psc.tile([128, S], F32, tag="big", name="scp")
                for nh in range(S // NHALF):
                    mm = nc.tensor.matmul(
                        scp[:, NHALF * nh:NHALF * (nh + 1)],
                        kaug[:, S * head + 128 * kb:S * head + 128 * (kb + 1)],
                        qaug[:, S * head + NHALF * nh:S * head + NHALF * (nh + 1)],
                        start=True, stop=True)
                    if state["first_sc"]:
                        state["first_sc"] = False
                        for cp in band_copy_insts:
                            add_dep_helper(mm.ins, cp.ins, sync=True,
                                           reason="scores read band rows")
                return scp

            def av_mms(av, head, kb, e):
                for nh in range(S // NHALF):
                    mm = nc.tensor.matmul(
                        av[:, NHALF * nh:NHALF * (nh + 1)],
                        v_sb[kb][:, (HD + 1) * head:(HD + 1) * (head + 1)],
                        e[:, NHALF * nh:NHALF * (nh + 1)],
                        start=(kb == 0), stop=(kb == NKB - 1))
                    if state["first_av"]:
                        state["first_av"] = False
                        for cp in vcopy_insts:
                            add_dep_helper(mm.ins, cp.ins, sync=True,
                                           reason="av reads v_sb")

            def norm(av, head, last=False):
                # the reciprocal input must sit at partition 0 (the custom
                # DVE op mishandles partition-offset inputs), hence drow
                drow = small.tile([1, S], F32, tag="drow", name="drow", bufs=2)
                if last:
                    # ScalarE is free at the tail; keep DVE for recip+mult
                    nc.scalar.copy(drow[:], av[HD:HD + 1, :])
                else:
                    nc.vector.tensor_copy(drow[:], av[HD:HD + 1, :])
                if not last:
                    # copy av to SBUF: frees the PSUM ring slot early so the
                    # next pair's av matmuls don't wait on this norm chain
                    avsb = small.tile([HD + 1, S], F32, tag="avsb",
                                      name="avsb", bufs=2)
                    nc.vector.tensor_copy(avsb[:], av[:])
                    avsrc = avsb[0:HD, :]
                else:
                    avsrc = av[0:HD, :]
                rrow = small.tile([1, S], F32, tag="recip", name="rrow", bufs=2)
                nc.vector.reciprocal_approx_fast(rrow[:], drow[:])
                rb = small.tile([64, S], F32, tag="rbcast", name="rb", bufs=2)
                nc.gpsimd.partition_broadcast(rb[:], rrow[:])
                chunk, row = head // 2, (head % 2) * 64
                nc.vector.tensor_tensor(
                    out_hT[chunk][row:row + 64, :], avsrc, rb[:],
                    op=mybir.AluOpType.mult)

            def proj_pass1(qb, scalar_copy=False):
                # chunks 0,1 of the projection, run during pair-2 attention
                pp1 = psc.tile([128, C], F32, tag="big", name="pp1")
                for ci in (0, 1):
                    nc.tensor.matmul(
                        pp1[:, 0:NHALF],
                        out_hT[ci][:, 128 * qb:128 * (qb + 1)],
                        wproj_sb[:, C * ci:C * ci + NHALF],
                        start=(ci == 0), stop=(ci == 1))
                    nc.tensor.matmul(
                        pp1[:, NHALF:C],
                        out_hT[ci][:, 128 * qb:128 * (qb + 1)],
                        wproj_sb[:, C * ci + NHALF:C * (ci + 1)],
                        start=(ci == 0), stop=(ci == 1))
                pp_sb = small.tile([128, C], F32, tag="pp_sb", name="pp_sb",
                                   bufs=4)
                if scalar_copy:
                    nc.scalar.copy(pp_sb[:], pp1[:])
                else:
                    nc.vector.tensor_copy(pp_sb[:], pp1[:])
                nc.sync.dma_start(out1[128 * qb:128 * (qb + 1), :], pp_sb[:])

            for p in range(HPC // 2):
                ha, hb = 2 * p, 2 * p + 1
                av_a = pav.tile([HD + 1, S], F32, tag="av", name="av_a")
                av_b = pav.tile([HD + 1, S], F32, tag="av", name="av_b")
                ea = eb = ebprev = None
                for kb in range(NKB + 1):
                    if kb < NKB:
                        sca = sc_mms(ha, kb)
                        scb = sc_mms(hb, kb)
                    if kb > 0:
                        av_mms(av_a, ha, kb - 1, ea)
                    if kb > 1:
                        av_mms(av_b, hb, kb - 2, ebprev)
                    if kb < NKB:
                        ea = et_pool.tile([128, S], BF16, tag="et", name="ea")
                        nc.scalar.activation(ea[:], sca[:], EXP)
                        ebprev = eb
                        eb = et_pool.tile([128, S], BF16, tag="et", name="eb")
                        nc.scalar.activation(eb[:], scb[:], EXP)
                    if p == 2 and 2 <= kb:
                        proj_pass1(kb - 2)
                norm(av_a, ha, last=(p == 2))
                av_mms(av_b, hb, NKB - 1, eb)
                if p == 2:
                    proj_pass1(6, scalar_copy=True)
                    proj_pass1(7, scalar_copy=True)
                norm(av_b, hb, last=(p == 2))

            # ---- chunk-2 projection after the last norm ----
            for qb in range(NQB):
                pp2 = psc.tile([128, C], F32, tag="big", name="pp2")
                nc.tensor.matmul(
                    pp2[:, 0:NHALF],
                    out_hT[2][:, 128 * qb:128 * (qb + 1)],
                    wproj_sb[:, 2 * C:2 * C + NHALF],
                    start=True, stop=True)
                nc.tensor.matmul(
                    pp2[:, NHALF:C],
                    out_hT[2][:, 128 * qb:128 * (qb + 1)],
                    wproj_sb[:, 2 * C + NHALF:3 * C],
                    start=True, stop=True)
                pp_sb2 = small.tile([128, C], F32, tag="pp_sb2", name="pp_sb2",
                                    bufs=4)
                (nc.scalar.copy if qb % 2 else nc.vector.tensor_copy)(
                    pp_sb2[:], pp2[:])
                nc.sync.dma_start(out2[128 * qb:128 * (qb + 1), :], pp_sb2[:])

    nc.compile()
    return nc


def shard_inputs(x, Wqkv, Wproj, rel_pos_h, rel_pos_w):
    """Build the 8 per-core input maps."""
    import ml_dtypes
    bf16 = ml_dtypes.bfloat16
    scale = HD ** (-0.5)
    x = np.asarray(x, dtype=np.float32)
    Wqkv = np.asarray(Wqkv, dtype=np.float32)
    Wproj = np.asarray(Wproj, dtype=np.float32)
    rhT = np.ascontiguousarray(np.asarray(rel_pos_h, np.float32).T).astype(bf16)
    rwT = np.ascontiguousarray(np.asarray(rel_pos_w, np.float32).T).astype(bf16)
    oh = np.zeros((64, S), np.float32)
    for khp in range(H):
        oh[khp, (31 - khp) * W:(31 - khp) * W + W] = 1.0
    for kwp in range(W):
        oh[32 + kwp, 31 - kwp::W] = 1.0
    oh64 = np.ascontiguousarray(np.tile(oh, (1, HPC))).astype(bf16)
    ones = np.ones((1, S), np.float32).astype(bf16)
    in_maps = []
    for core in range(NCORES):
        b = core // 2
        h0 = (core % 2) * HPC
        xb = x[b].reshape(S, C)
        xT = np.ascontiguousarray(xb.T).astype(bf16)
        wq = Wqkv[:, h0 * HD:(h0 + HPC) * HD]
        wk = Wqkv[:, C + h0 * HD:C + (h0 + HPC) * HD] * scale
        wqk = np.ascontiguousarray(np.concatenate([wq, wk], axis=1)).astype(bf16)
        wv = np.ascontiguousarray(
            Wqkv[:, 2 * C + h0 * HD:2 * C + (h0 + HPC) * HD]).astype(bf16)
        wp = np.ascontiguousarray(Wproj[h0 * HD:(h0 + HPC) * HD, :]).astype(bf16)
        in_maps.append({"xT": xT, "wqk": wqk, "wv": wv, "wproj": wp,
                        "rhT": rhT, "rwT": rwT, "oh64": oh64, "ones": ones})
    return in_maps


def combine(results, bproj):
    """Sum the per-core partials into the full (B, H, W, C) output."""
    bproj = np.asarray(bproj, dtype=np.float32)
    out = np.empty((B, H, W, C), dtype=np.float32)
    for b in range(B):
        acc = bproj.copy()
        for core in (2 * b, 2 * b + 1):
            acc = acc + results[core]["out1"] + results[core]["out2"]
        out[b] = acc.reshape(H, W, C)
    return out


_NC_CACHE = {}


def kernel(x, Wqkv, Wproj, bproj, rel_pos_h, rel_pos_w):
    if "nc" not in _NC_CACHE:
        _NC_CACHE["nc"] = build_program()
    nc = _NC_CACHE["nc"]
    in_maps = shard_inputs(x, Wqkv, Wproj, rel_pos_h, rel_pos_w)
    res = run_bass_kernel_spmd(nc, in_maps, list(range(NCORES)))
    return combine(res.results, bproj)
